# revision 20
# baseline (speedup 1.0000x reference)
"""Trainium2 Bass kernel: 16-head causal attention (T=4096, C=1024) on 8 NeuronCores.

Sharding: heads across cores (2 heads = 128 channels per core).
 - Each core computes Q,K (transposed layout [ch, T]) and V for its 2 heads
   from the full x; no comm until the output projection.
 - Scores are computed TRANSPOSED (k on partitions, q on free dim) so the
   P@V matmul needs no transposes and the softmax denominator comes free
   via a ones-column appended to V. The two heads' score matmuls (K=64)
   auto-row-tile into the two halves of the PE array and run concurrently.
 - V is projected transposed (like Q/K, full-width N=512 matmuls) and
   flipped to the [token, ch] layout the PV matmul needs with PE-mode
   transposes — much cheaper than 32 N=128 matmuls per window.
 - Scores for this input distribution are bounded (|s*scale| < ~3), so
   softmax is computed without max-subtraction (mathematically identical).
 - Causal masking: on the 4 diagonal key blocks of each query window the
   fully-masked query columns are skipped in the score/PV matmuls and the
   exp; the partial triangle is zeroed by a 0/1 bf16 mask after exp.
 - Output projection is computed TRANSPOSED (out.T [128 out-ch, T]) so the
   gathered activations are consumed in their natural [ch, tok] layout with
   N=512 matmuls; sharded by OUTPUT channels; host transposes at the end.
 - AllGather is chunked per 512-query window; gather loads and the output
   projection are interleaved into the attention loop a few windows behind,
   so almost nothing runs after the last window's gather.
"""

import os
import sys

import numpy as np

for _p in ("/opt/trn_rl_repo",):
    if os.path.isdir(_p) and _p not in sys.path:
        sys.path.insert(0, _p)

import ml_dtypes

T = 4096
C = 1024
H = 16
DH = 64
R = 8           # cores
HL = H // R     # heads per core
CH = C // R     # channels per core (2 heads * 64)
QW = 512        # query window (free dim of score tiles)
KB = 128        # key block (partition dim of score tiles)
NQW = T // QW   # 8
NKB = T // KB   # 32
NCH = C // 128  # contraction chunks over C
SCALE = float(C) ** -0.5
BF16 = ml_dtypes.bfloat16
# window -> list of output projections emitted (spread) inside that window.
# Gathers take 15-45us and the gather-output loads ~20us, so out-proj work
# only starts once its inputs are surely in SBUF.
OPROJ_AT = {5: [0, 1, 2], 6: [3, 4], 7: [5]}
# window -> gather-output SBUF loads issued on the gpsimd ring at the END
# of that window (after the window's own gather is issued; the sync ring
# carrying gin/out stores never queues behind these 20us transfers).
FL_AT_END = {2: [0], 3: [1], 4: [2], 5: [3, 4], 6: [5]}
FL_AT_START = {7: [6]}

LAST_RESULT = None  # BassKernelResults of the most recent run (for test harness)

_nc = None


def _build():
    import concourse.mybir as mybir
    import concourse.tile as tile
    from concourse import bacc
    from concourse.masks import make_identity

    f32 = mybir.dt.float32
    bf16 = mybir.dt.bfloat16
    EXP = mybir.ActivationFunctionType.Exp

    nc = bacc.Bacc("TRN2", target_bir_lowering=False, num_devices=R)

    xT_d = nc.declare_dram_parameter("xT", [C, T], bf16, isOutput=False)
    wq_d = nc.declare_dram_parameter("wqT", [128, NCH * CH], bf16, isOutput=False)
    wk_d = nc.declare_dram_parameter("wkT", [128, NCH * CH], bf16, isOutput=False)
    wv_d = nc.declare_dram_parameter("wvT", [128, NCH * CH], bf16, isOutput=False)
    wp_d = nc.declare_dram_parameter("wpT", [128, NCH * CH], bf16, isOutput=False)
    bq_d = nc.declare_dram_parameter("bqc", [CH, 1], f32, isOutput=False)
    bk_d = nc.declare_dram_parameter("bkc", [CH, 1], f32, isOutput=False)
    bv_d = nc.declare_dram_parameter("bvc", [CH, 1], f32, isOutput=False)
    bp_d = nc.declare_dram_parameter("bpc", [CH, 1], f32, isOutput=False)
    cm_d = nc.declare_dram_parameter("cmask", [128, 4 * QW], bf16, isOutput=False)
    out_d = nc.declare_dram_parameter("out", [CH, T], f32, isOutput=True)

    with tile.TileContext(nc, num_cores=R) as tc:
        with (
            tc.tile_pool(name="const", bufs=1) as constp,
            tc.tile_pool(name="big", bufs=1) as bigp,
            tc.tile_pool(name="dram", bufs=1, space="DRAM") as dramp,
        ):
            # persistent SBUF tensors
            xs = bigp.tile([128, NCH * T], bf16)           # x.T chunks, 8 MB
            qt_s = bigp.tile([128, T], bf16)               # Q.T  [2h*64, T]
            kt_s = bigp.tile([128, T], bf16)               # K.T
            vb_s = bigp.tile([128, HL * NKB * 65], bf16)   # V tiles [128t, 64]+ones col
            wq_s = constp.tile([128, NCH * CH], bf16)
            wk_s = constp.tile([128, NCH * CH], bf16)
            wv_s = constp.tile([128, NCH * CH], bf16)
            wp_s = constp.tile([128, NCH * CH], bf16)
            bq_s = constp.tile([CH, 1], f32)
            bk_s = constp.tile([CH, 1], f32)
            bv_s = constp.tile([CH, 1], f32)
            bp_s = constp.tile([CH, 1], f32)
            cm_s = constp.tile([128, 4 * QW], bf16)
            ident = constp.tile([128, 128], bf16)

            # startup DMA order: what window 0 needs first (biases, Wq/Wk,
            # x tokens 0..512, Wv); later x T-eighths are issued inside the
            # window loop so their issue cost never delays gather stores.
            # Each eighth is ONE 3D-AP DMA (issue cost on the sync engine is
            # ~0.6us per dma_start, so fewer, bigger issues).
            for b_s, b_d in ((bq_s, bq_d), (bk_s, bk_d), (bv_s, bv_d), (bp_s, bp_d)):
                nc.sync.dma_start(b_s[:], b_d[:])
            nc.sync.dma_start(wq_s[:], wq_d[:])
            nc.sync.dma_start(wk_s[:], wk_d[:])

            xs3 = xs[:].rearrange("p (c t) -> p c t", c=NCH)
            xT3 = xT_d[:].rearrange("(c p) t -> p c t", c=NCH)

            def load_x_eighth(tq):
                nc.sync.dma_start(
                    xs3[:, :, tq * QW:(tq + 1) * QW],
                    xT3[:, :, tq * QW:(tq + 1) * QW],
                )

            load_x_eighth(0)
            nc.sync.dma_start(wv_s[:], wv_d[:])
            load_x_eighth(1)
            nc.sync.dma_start(cm_s[:], cm_d[:])
            nc.sync.dma_start(wp_s[:], wp_d[:])
            load_x_eighth(2)

            make_identity(nc, ident[:])
            nc.gpsimd.memset(vb_s[:], 1.0)

            gouts = [None] * NQW
            fls = [None] * NQW
            with (
                tc.tile_pool(name="stp", bufs=2, space="PSUM") as stp,
                tc.tile_pool(name="otp", bufs=2, space="PSUM") as otp,
                tc.tile_pool(name="vap", bufs=2, space="PSUM") as vap,
                tc.tile_pool(name="pp", bufs=6) as pp,
                tc.tile_pool(name="aop", bufs=2) as aop,
                tc.tile_pool(name="vtp", bufs=2) as vtp,
                tc.tile_pool(name="flp", bufs=2) as flp,
                tc.tile_pool(name="fop", bufs=2) as fop,
                tc.tile_pool(name="smallp", bufs=4) as smallp,
            ):
                # ---- interleavable PE work groups ----
                def make_qk_groups(tw):
                    groups = []
                    for w_s, b_s, dst, nm in (
                        (wq_s, bq_s, qt_s, "q"),
                        (wk_s, bk_s, kt_s, "k"),
                    ):
                        box = {}

                        def g0(w_s=w_s, box=box, tw=tw, nm=nm):
                            acc = vap.tile(
                                [128, QW], f32, tag="vacc", name=f"acc{nm}{tw}"
                            )
                            box["acc"] = acc
                            for c in range(4):
                                nc.tensor.matmul(
                                    acc[:],
                                    w_s[:, c * CH:(c + 1) * CH],
                                    xs[:, c * T + tw * QW: c * T + tw * QW + QW],
                                    start=(c == 0),
                                    stop=False,
                                    skip_group_check=True,
                                )

                        def g1(w_s=w_s, b_s=b_s, dst=dst, box=box, tw=tw):
                            acc = box["acc"]
                            for c in range(4, NCH):
                                nc.tensor.matmul(
                                    acc[:],
                                    w_s[:, c * CH:(c + 1) * CH],
                                    xs[:, c * T + tw * QW: c * T + tw * QW + QW],
                                    start=False,
                                    stop=(c == NCH - 1),
                                    skip_group_check=True,
                                )
                            nc.vector.tensor_scalar_add(
                                dst[:, tw * QW:(tw + 1) * QW], acc[:], b_s[:]
                            )

                        groups += [g0, g1]
                    return groups

                def make_v_groups(tw):
                    box = {}

                    def g0():
                        acc = vap.tile([128, QW], f32, tag="vacc", name=f"vacc{tw}")
                        box["acc"] = acc
                        for c in range(4):
                            nc.tensor.matmul(
                                acc[:],
                                wv_s[:, c * CH:(c + 1) * CH],
                                xs[:, c * T + tw * QW: c * T + tw * QW + QW],
                                start=(c == 0),
                                stop=False,
                                skip_group_check=True,
                            )

                    def g1(tw=tw):
                        acc = box["acc"]
                        for c in range(4, NCH):
                            nc.tensor.matmul(
                                acc[:],
                                wv_s[:, c * CH:(c + 1) * CH],
                                xs[:, c * T + tw * QW: c * T + tw * QW + QW],
                                start=False,
                                stop=(c == NCH - 1),
                                skip_group_check=True,
                            )
                        vt = vtp.tile([128, QW], bf16, tag="vt")
                        box["vt"] = vt
                        nc.vector.tensor_scalar_add(vt[:], acc[:], bv_s[:])

                    def g2(tw=tw):
                        # flip [ch, tok] -> [tok, ch] with PE-mode transposes
                        vt = box["vt"]
                        tp = vap.tile([128, QW], bf16, tag="vacc", name=f"tp{tw}")
                        for j in range(4):
                            nc.tensor.transpose(
                                tp[:, j * 128:(j + 1) * 128],
                                vt[:, j * 128:(j + 1) * 128],
                                ident[:],
                            )
                        for j in range(4):
                            tt = tw * 4 + j
                            for h in range(HL):
                                base = (h * NKB + tt) * 65
                                nc.vector.tensor_copy(
                                    vb_s[:, base:base + 64],
                                    tp[:, j * 128 + h * 64: j * 128 + h * 64 + 64],
                                )

                    return [g0, g1, g2]

                def emit_fl(p, split=False):
                    fl = flp.tile([128, NCH * QW], bf16, tag="fl", name=f"fl{p}")
                    fls[p] = fl
                    fl3 = fl[:].rearrange("p (c m) -> p c m", c=NCH)
                    go3 = gouts[p][:].rearrange("(c p) m -> p c m", c=NCH)
                    if split:  # tail: two rings pull halves in parallel
                        h = NCH // 2
                        nc.gpsimd.dma_start(fl3[:, :h], go3[:, :h])
                        nc.sync.dma_start(fl3[:, h:], go3[:, h:])
                    else:
                        nc.gpsimd.dma_start(fl3, go3)

                def emit_oproj(p):
                    fl = fls[p]
                    po = vap.tile([128, QW], f32, tag="vacc", name=f"po{p}")
                    for c in range(NCH):
                        nc.tensor.matmul(
                            po[:],
                            wp_s[:, c * CH:(c + 1) * CH],
                            fl[:, c * QW:(c + 1) * QW],
                            start=(c == 0),
                            stop=(c == NCH - 1),
                        )
                    fo = fop.tile([128, QW], f32, tag="fo")
                    nc.vector.tensor_scalar_add(fo[:], po[:], bp_s[:])
                    nc.sync.dma_start(out_d[:, p * QW:(p + 1) * QW], fo[:])

                # prime the scalar engine's EXP table load (~2.7us) before
                # the first real activation needs it
                prime = smallp.tile([CH, 1], f32, tag="prime")
                nc.scalar.activation(prime[:], bq_s[:], EXP, bias=0.0, scale=0.0)

                # window 0's Q/K/V before the loop
                for g in make_qk_groups(0):
                    g()
                for g in make_v_groups(0):
                    g()

                for qw in range(NQW):
                    if qw + 3 < NQW:
                        load_x_eighth(qw + 3)
                    for p in FL_AT_START.get(qw, ()):
                        emit_fl(p)
                    pending = []
                    if qw + 1 < NQW:
                        pending += make_qk_groups(qw + 1)
                        pending += make_v_groups(qw + 1)
                    for p in OPROJ_AT.get(qw, ()):
                        pending.append(lambda p=p: emit_oproj(p))
                    npend0 = max(1, len(pending))
                    nkb = 4 * (qw + 1)  # causal: key blocks 0 .. end of window
                    npairs = nkb // 2
                    ots = [
                        otp.tile([65, QW], f32, tag="ot", name=f"ot{qw}_{h}")
                        for h in range(HL)
                    ]
                    for pair_i, kb0 in enumerate(range(0, nkb, 2)):
                        kbs = (kb0, kb0 + 1)
                        q0s = [max(0, kb * KB - qw * QW) for kb in kbs]
                        sts = []
                        for kb, q0 in zip(kbs, q0s):
                            st = stp.tile([128, 2 * QW], f32, tag="st")
                            for h in range(HL):
                                nc.tensor.matmul(
                                    st[:, h * QW + q0:(h + 1) * QW],
                                    kt_s[h * 64:(h + 1) * 64, kb * KB:(kb + 1) * KB],
                                    qt_s[h * 64:(h + 1) * 64,
                                         qw * QW + q0:(qw + 1) * QW],
                                    start=True,
                                    stop=True,
                                )
                            sts.append(st)
                        ps = []
                        for st, kb, q0 in zip(sts, kbs, q0s):
                            p = pp.tile([128, 2 * QW], bf16, tag="p")
                            if q0 > 0:
                                for h in range(HL):
                                    nc.scalar.activation(
                                        p[:, h * QW + q0:(h + 1) * QW],
                                        st[:, h * QW + q0:(h + 1) * QW],
                                        EXP, bias=0.0, scale=SCALE,
                                    )
                                j = (kb * KB - qw * QW) // KB
                                for h in range(HL):
                                    nc.vector.tensor_mul(
                                        p[:, h * QW + q0:(h + 1) * QW],
                                        p[:, h * QW + q0:(h + 1) * QW],
                                        cm_s[:, j * QW + q0:(j + 1) * QW],
                                    )
                            else:
                                nc.scalar.activation(
                                    p[:], st[:], EXP, bias=0.0, scale=SCALE
                                )
                                if kb * KB == qw * QW:  # diagonal block j==0
                                    for h in range(HL):
                                        nc.vector.tensor_mul(
                                            p[:, h * QW:(h + 1) * QW],
                                            p[:, h * QW:(h + 1) * QW],
                                            cm_s[:, 0:QW],
                                        )
                            ps.append(p)
                        for p, kb, q0 in zip(ps, kbs, q0s):
                            for h in range(HL):
                                base = (h * NKB + kb) * 65
                                nc.tensor.matmul(
                                    ots[h][:, q0:QW],
                                    vb_s[:, base:base + 65],
                                    p[:, h * QW + q0:(h + 1) * QW],
                                    start=(kb == 0),
                                    stop=(kb == nkb - 1),
                                    skip_group_check=True,
                                )
                        # spread pending projection groups evenly over the pairs
                        want_left = (npairs - 1 - pair_i) * npend0 // npairs
                        while pending and len(pending) > want_left:
                            pending.pop(0)()
                    while pending:
                        pending.pop(0)()
                    # move OT off PSUM fast (both heads first, freeing both
                    # PSUM slots before the slow reciprocals run)
                    ao = aop.tile([128, QW], bf16, tag="ao")
                    osbs = []
                    for h in range(HL):
                        osb = smallp.tile([65, QW], f32, tag="osb", name=f"osb{h}")
                        nc.vector.tensor_copy(osb[:], ots[h][:])
                        osbs.append(osb)
                    for h in range(HL):
                        osb = osbs[h]
                        den = smallp.tile([1, QW], f32, tag="den")
                        nc.vector.tensor_copy(den[:], osb[64:65, :])
                        rec = smallp.tile([1, QW], f32, tag="rec")
                        nc.vector.reciprocal_approx_fast(rec[:], den[:])
                        rb = smallp.tile([64, QW], f32, tag="rb")
                        nc.gpsimd.partition_broadcast(rb[:], rec[:])
                        nc.vector.tensor_mul(
                            ao[h * 64:(h + 1) * 64, :], osb[0:64, :], rb[:]
                        )
                    gin = dramp.tile([128, QW], bf16, tag=f"gin{qw}")
                    nc.sync.dma_start(gin[:], ao[:])
                    gout = dramp.tile(
                        [R * 128, QW], bf16, tag=f"gout{qw}", addr_space="Shared"
                    )
                    nc.gpsimd.collective_compute(
                        "AllGather",
                        mybir.AluOpType.bypass,
                        ins=[gin.opt()],
                        outs=[gout.opt()],
                        replica_groups=[list(range(R))],
                    )
                    gouts[qw] = gout
                    for p in FL_AT_END.get(qw, ()):
                        emit_fl(p)

                # drain the remaining windows' gathers + output projections
                emit_oproj(6)
                emit_fl(7, split=True)
                emit_oproj(7)

    nc.compile()
    return nc


def _get_nc():
    global _nc
    if _nc is None:
        _nc = _build()
    return _nc


def _chunked_wT(w):
    # W_loc [CH, C] -> W_loc.T [C, CH] -> chunk layout [128, NCH*CH]
    wt = np.ascontiguousarray(w.T).reshape(NCH, 128, CH)
    return np.ascontiguousarray(
        wt.transpose(1, 0, 2).reshape(128, NCH * CH)
    ).astype(BF16)


def _causal_masks():
    kl = np.arange(KB)[:, None]
    ql = np.arange(QW)[None, :]
    cols = []
    for j in range(4):
        cols.append((kl + j * KB <= ql).astype(np.float32))
    return np.concatenate(cols, axis=1).astype(BF16)  # [128, 2048] of 0/1


def kernel(x, Wq, bq, Wk, bk, Wv, bv, Wp, bp):
    global LAST_RESULT
    from concourse.bass_utils import run_bass_kernel_spmd

    x = np.asarray(x, np.float32)
    Wq = np.asarray(Wq, np.float32)
    Wk = np.asarray(Wk, np.float32)
    Wv = np.asarray(Wv, np.float32)
    Wp = np.asarray(Wp, np.float32)
    bq = np.asarray(bq, np.float32)
    bk = np.asarray(bk, np.float32)
    bv = np.asarray(bv, np.float32)
    bp = np.asarray(bp, np.float32)

    xT16 = np.ascontiguousarray(x.T).astype(BF16)
    cmask = _causal_masks()

    in_maps = []
    for r in range(R):
        sl = slice(r * CH, (r + 1) * CH)
        in_maps.append(
            {
                "xT": xT16,
                "wqT": _chunked_wT(Wq[sl, :]),
                "wkT": _chunked_wT(Wk[sl, :]),
                "wvT": _chunked_wT(Wv[sl, :]),
                "wpT": _chunked_wT(Wp[sl, :]),
                "bqc": np.ascontiguousarray(bq[sl][:, None]),
                "bkc": np.ascontiguousarray(bk[sl][:, None]),
                "bvc": np.ascontiguousarray(bv[sl][:, None]),
                "bpc": np.ascontiguousarray(bp[sl][:, None]),
                "cmask": cmask,
            }
        )

    nc = _get_nc()
    res = run_bass_kernel_spmd(nc, in_maps, core_ids=list(range(R)))
    LAST_RESULT = res
    out = np.empty((T, C), np.float32)
    for r in range(R):
        out[:, r * CH:(r + 1) * CH] = np.asarray(
            res.results[r]["out"], np.float32
        ).T
    return out
